# revision 33
# baseline (speedup 1.0000x reference)
"""GNN message passing on 8 TRN2 cores — dense-packed gather variant.

Like kernel.py, but tokens are packed DENSELY inside each (block-group of 4,
segment) span: no per-(block, segment) chunk padding (which cost ~22% extra
gather bytes). Chunks may straddle block boundaries; each (chunk, touched
block) pair becomes a matmul "variant" with its own one-hot column set (other
blocks' tokens masked to slot 255). The variant schedule is the union over
the 8 cores (all cores share one program); a variant that a core doesn't
need has an all-255 slot column there and contributes zero.
"""

import numpy as np

N_NODES = 100000
N_EDGES = 1250000
D = 64
NCORES = 8
SHARD = 12500
NBLK = 98
NSEG = 4
SEGSZ = 25000
NCELL = NBLK * NSEG
OUT_ROWS = NBLK * 128  # 12544
BGSZ = 4
NBG = -(NBLK // -BGSZ)  # 25
NGRP = NBG * NSEG       # 100 gather groups

RCG = 8      # msg ring, in groups (multiple of the 4 SWDGE queues)
SEL = 64     # one-hot ring, in variants
G = 8        # variants per DVE compare instruction
NPS = 8      # psum tiles
NSTG = 4     # output stage ring, in blocks


def host_prep(x, edge_index):
    row = np.asarray(edge_index[0], dtype=np.int64)
    col = np.asarray(edge_index[1], dtype=np.int64)
    core = row // SHARD
    rloc = row - core * SHARD
    blk = rloc >> 7
    seg = col // SEGSZ
    cloc = (col - seg * SEGSZ).astype(np.int16)
    grp = (blk // BGSZ) * NSEG + seg           # gather group id (bg, s)

    gcount = np.zeros((NCORES, NGRP), dtype=np.int64)
    np.add.at(gcount, (core, grp), 1)
    nch_g = -(np.maximum(np.max(gcount, axis=0), 1) // -128)  # chunks per group
    cum = np.concatenate([[0], np.cumsum(nch_g)]).astype(np.int64)
    TC = int(cum[-1])
    T = TC * 128
    off = cum[:-1] * 128                       # token offset per group

    # per-core token streams, densely packed, sorted by (group, block, rloc)
    import ml_dtypes
    x = np.ascontiguousarray(np.asarray(x, dtype=np.float32))
    core_tok_rloc = []                         # per core: rloc per token (pad=BIG)
    gws = []
    BIG = 1 << 30
    for k in range(NCORES):
        m = core == k
        gk = grp[m]
        order = np.lexsort((rloc[m], gk))
        gk = gk[order]
        clk = cloc[m][order]
        rlk = rloc[m][order]
        cnts = np.bincount(gk, minlength=NGRP)
        starts = np.concatenate([[0], np.cumsum(cnts)])[:-1]
        within = np.arange(len(gk)) - starts[gk]
        tok = off[gk] + within
        gidx = np.full(T, 0, dtype=np.int16)   # pads gather row 0
        rl = np.full(T, BIG, dtype=np.int64)
        gidx[tok] = clk
        rl[tok] = rlk
        gws.append(np.tile(gidx.reshape(-1, 16).T, (8, 1)).copy())
        core_tok_rloc.append(rl)

    # variant schedule: union over cores of blocks present per chunk
    bmin = np.full(TC, 1 << 20, dtype=np.int64)
    bmax = np.full(TC, -1, dtype=np.int64)
    for k in range(NCORES):
        bt = core_tok_rloc[k] >> 7             # block per token (pads huge)
        btc = bt.reshape(TC, 128)
        valid = btc < NBLK
        btc_min = np.where(valid, btc, 1 << 20).min(axis=1)
        btc_max = np.where(valid, btc, -1).max(axis=1)
        bmin = np.minimum(bmin, btc_min)
        bmax = np.maximum(bmax, btc_max)

    variants = []                              # (chunk, block)
    for c in range(TC):
        if bmax[c] >= 0:
            for b in range(int(bmin[c]), int(bmax[c]) + 1):
                variants.append((c, b))
    # ensure every block has at least one variant
    have = {b for _, b in variants}
    for b in range(NBLK):
        if b not in have:
            g0 = (b // BGSZ) * NSEG            # its (bg, s=0) group
            variants.append((int(cum[g0]), b))
    variants.sort()
    var_c = np.array([c for c, _ in variants], dtype=np.int64)
    var_b = np.array([b for _, b in variants], dtype=np.int64)
    NV = len(variants)

    # per-core slot table [128, NV]: token p of chunk var_c relative to var_b
    per_core = []
    for k in range(NCORES):
        rl = core_tok_rloc[k].reshape(TC, 128)
        sv = rl[var_c] - var_b[:, None] * 128  # [NV, 128]
        sv = np.where((sv >= 0) & (sv < 128), sv, 255).astype(np.int32)
        sw = np.ascontiguousarray(sv.T.astype(ml_dtypes.bfloat16))
        per_core.append({"x": x, "gidx": gws[k], "slot": sw})

    return per_core, nch_g, cum, var_c, var_b, T, TC, NV


def build_bass(nch_g, cum, var_c, var_b, T, TC, NV):
    import concourse.bacc as bacc
    import concourse.mybir as mybir
    from concourse.bass import AP
    import contextlib

    f32, bf16, i16 = mybir.dt.float32, mybir.dt.bfloat16, mybir.dt.int16

    maxspan = int(np.max(nch_g))
    chunk_grp = np.zeros(TC, dtype=np.int64)
    for g_ in range(NGRP):
        chunk_grp[cum[g_]:cum[g_ + 1]] = g_

    # per block: first/last variant index; per group: last variant index
    blk_first_v = {}
    blk_last_v = {}
    for v in range(NV):
        b = int(var_b[v])
        if b not in blk_first_v:
            blk_first_v[b] = v
        blk_last_v[b] = v
    grp_last_v = np.full(NGRP, -1, dtype=np.int64)
    for v in range(NV):
        grp_last_v[chunk_grp[var_c[v]]] = v

    nc = bacc.Bacc(None, target_bir_lowering=False, debug=False,
                   num_swdge_queues=4)
    x = nc.dram_tensor("x", [N_NODES, D], f32, kind="ExternalInput")
    gidx = nc.dram_tensor("gidx", [128, T // 16], i16, kind="ExternalInput")
    slot = nc.dram_tensor("slot", [128, NV], bf16, kind="ExternalInput")
    out = nc.dram_tensor("out", [OUT_ROWS, D], f32, kind="ExternalOutput")

    last_wait = {}

    def wge(eng, sem, val):
        if val <= 0:
            return
        key = (id(eng), id(sem))
        if last_wait.get(key, 0) >= val:
            return
        eng.wait_ge(sem, val)
        last_wait[key] = val

    with (
        nc.sbuf_tensor([128, T // 16], i16) as gi_sb,
        nc.sbuf_tensor([128, NV], bf16) as slot_sb,
        nc.sbuf_tensor([128, 128], bf16) as iota_sb,
        nc.sbuf_tensor([128, RCG * maxspan * 64], f32) as msg32,
        nc.sbuf_tensor([128, RCG * maxspan * 64], bf16) as msg16,
        nc.sbuf_tensor([128, SEL * 128], bf16) as selT,
        nc.sbuf_tensor([128, NSTG * 64], f32) as stage,
        nc.semaphore("lsem") as lsem,
        nc.semaphore("lsemB") as lsemB,
        nc.semaphore("lsemC") as lsemC,
        nc.semaphore("isem") as isem,
        nc.semaphore("csem") as csem,
        nc.semaphore("vsem") as vsem,
        nc.semaphore("pesem") as pesem,
        nc.semaphore("cpsem") as cpsem,
        contextlib.ExitStack() as stack,
        nc.Block(no_gpsimd_drain=True) as block,
    ):
        gsems = [stack.enter_context(nc.semaphore(f"gsem{i2}"))
                 for i2 in range(RCG)]
        osems = [stack.enter_context(nc.semaphore(f"osem{i2}"))
                 for i2 in range(NSTG)]
        psums = [stack.enter_context(nc.psum_tensor(f"ps{i2}", [128, 64], f32))
                 for i2 in range(NPS)]

        def chunk_col(c):
            g_ = int(chunk_grp[c])
            return (g_ % RCG) * maxspan * 64 + (c - int(cum[g_])) * 64

        # first slice: exactly what the first ring-fill of gathers needs
        IH0 = (int(cum[min(RCG, NGRP)]) * 128) // 16
        IH = max(IH0 + 1, (T // 16) // 2)

        @block.sync
        def _(sy):
            sy.dma_start(out=gi_sb[:, :IH0], in_=gidx[:, :IH0]).then_inc(lsem, 16)
            sy.dma_start(out=gi_sb[:, IH0:IH], in_=gidx[:, IH0:IH]).then_inc(lsemB, 16)
            sy.dma_start(out=gi_sb[:, IH:], in_=gidx[:, IH:]).then_inc(lsemC, 16)
            sy.dma_start(out=slot_sb[:], in_=slot[:]).then_inc(isem, 16)

        @block.gpsimd
        def _(g):
            g.iota(iota_sb[:, :], [[1, 128]], channel_multiplier=0,
                   allow_small_or_imprecise_dtypes=True).then_inc(isem, 1)
            for gi_ in range(NGRP):
                s = gi_ % NSEG
                span = int(nch_g[gi_])
                tokend = (int(cum[gi_]) + span) * 128
                if tokend // 16 <= IH0:
                    wge(g, lsem, 16)
                elif tokend // 16 <= IH:
                    wge(g, lsemB, 16)
                else:
                    wge(g, lsemC, 16)
                wge(g, csem, gi_ - RCG + 1)
                rcol = (gi_ % RCG) * maxspan * 64
                buf = msg32[:, rcol:rcol + span * 64]
                tok0 = int(cum[gi_]) * 128
                g.dma_gather(
                    out_ap=buf.rearrange("p (k dd) -> p k dd", dd=D),
                    in_ap=x[s * SEGSZ:(s + 1) * SEGSZ, :],
                    idxs_ap=gi_sb[:, tok0 // 16:(tok0 + span * 128) // 16],
                    num_idxs=span * 128,
                    num_idxs_reg=span * 128,
                    elem_size=D,
                    single_packet=False,
                    queue_num=gi_ % 4,
                ).then_inc(gsems[gi_ % RCG], 16)

        @block.scalar
        def _(se):
            for gi_ in range(NGRP):
                span = int(nch_g[gi_])
                wge(se, gsems[gi_ % RCG], 16 * (gi_ // RCG + 1))
                if gi_ >= RCG:
                    wge(se, pesem, int(grp_last_v[gi_ - RCG]) + 1)
                rcol = (gi_ % RCG) * maxspan * 64
                se.copy(out=msg16[:, rcol:rcol + span * 64],
                        in_=msg32[:, rcol:rcol + span * 64]).then_inc(csem)

        @block.vector
        def _(ve):
            ve.wait_ge(isem, 17)
            nvg = -(NV // -G)
            events = []
            for vg in range(nvg):
                events.append((vg, 0, "cmp", vg))
            for b in range(NBLK):
                gb = blk_last_v[b] // G
                events.append((gb, 1, "copy", b))
            events.sort(key=lambda e: (e[0], e[1]))
            for _, _, kind, v in events:
                if kind == "cmp":
                    v0 = v * G
                    gg = min(G, NV - v0)
                    wge(ve, pesem, v0 + gg - SEL)
                    out_ap = AP(selT, (v0 % SEL) * 128,
                                [[SEL * 128, 128], [128, gg], [1, 128]])
                    in0 = AP(slot_sb, v0, [[NV, 128], [1, gg], [0, 128]])
                    in1 = AP(iota_sb, 0, [[128, 128], [0, gg], [1, 128]])
                    ve.tensor_tensor(out_ap, in0, in1,
                                     mybir.AluOpType.is_equal).then_inc(vsem)
                else:
                    b = v
                    wge(ve, pesem, blk_last_v[b] + 1)
                    if b >= NSTG:
                        wge(ve, osems[b % NSTG], 16 * (b // NSTG))
                    ve.tensor_copy(out=stage[:, (b % NSTG) * 64:(b % NSTG + 1) * 64],
                                   in_=psums[b % NPS][:, :]).then_inc(cpsem)

        @block.tensor
        def _(te):
            for v in range(NV):
                c = int(var_c[v])
                b = int(var_b[v])
                wge(te, csem, int(chunk_grp[c]) + 1)
                wge(te, vsem, v // G + 1)
                start = (v == blk_first_v[b])
                if start:
                    wge(te, cpsem, b - NPS + 1)
                cc = chunk_col(c)
                te.matmul(
                    psums[b % NPS][:, :],
                    selT[:, (v % SEL) * 128:(v % SEL + 1) * 128],
                    msg16[:, cc:cc + 64],
                    start=start,
                    stop=(v == blk_last_v[b]),
                    skip_group_check=True,
                ).then_inc(pesem)

        @block.sync
        def _(sy):
            for b in range(NBLK):
                wge(sy, cpsem, b + 1)
                sy.dma_start(
                    out=out[b * 128:(b + 1) * 128, :],
                    in_=stage[:, (b % NSTG) * 64:(b % NSTG + 1) * 64],
                ).then_inc(osems[b % NSTG], 16)
            for i2 in range(NSTG):
                n_i = NBLK // NSTG + (1 if i2 < NBLK % NSTG else 0)
                sy.wait_ge(osems[i2], 16 * n_i)

    nc.compile()
    return nc


def run_spmd(nc, per_core, trace=False):
    from concourse.bass_utils import run_bass_kernel_spmd
    return run_bass_kernel_spmd(
        nc, per_core, core_ids=list(range(len(per_core))), trace=trace
    )


def kernel(x, edge_index, _trace=False, _return_results=False):
    x = np.asarray(x, dtype=np.float32)
    per_core, nch_g, cum, var_c, var_b, T, TC, NV = host_prep(x, edge_index)
    nc = build_bass(nch_g, cum, var_c, var_b, T, TC, NV)
    res = run_spmd(nc, per_core, trace=_trace)
    out = np.concatenate(
        [res.results[k]["out"][:SHARD] for k in range(NCORES)], axis=0)
    if _return_results:
        return out, res
    return out
